# revision 38
# baseline (speedup 1.0000x reference)
"""Ball-query KNN (BQWarp) on 8 Trainium2 NeuronCores.

Problem: N=32768 query points, G=4096 database points (flattened p_grid),
radius 0.25, K=10 neighbors (first-K in database-index order).

Strategy (data-parallel over queries, grid replicated per core):
  - Each core handles 4096 queries as 32 tiles of 128 (partition dim).
  - TensorE: d2'[q,g] = -2 q.g + |g|^2  via a 4-row contraction matmul.
  - DVE scalar_tensor_tensor: score = (d2' <= R^2 - |q|^2) * (G - g)
    so larger score = lower database index among in-radius points.
  - The db axis is host-permuted into 8 interleaved segments (g mod 8), so
    each segment sees ~1/8th of any query's neighbors.  Per segment a DVE
    max8 yields its top-8 scores; a 64-wide merge (max8 / filter / max8)
    yields the global top-16.  This is exact unless one segment held >=9 of
    the true top-16, which the kernel detects (segment 8th value > merged
    16th value) and flags; flagged queries (~1e-4 of them) are recomputed
    exactly on the host with the reference semantics.
  - Host decodes idx = G - score (score > 0) and gathers coordinates.
"""

import sys

sys.path.insert(0, "/opt/trn_rl_repo")

import numpy as np

import concourse.bass as bass
import concourse.mybir as mybir
from concourse import bacc
from concourse import tile
from concourse.bass_utils import run_bass_kernel_spmd

GRID = (32, 16, 8)
G = GRID[0] * GRID[1] * GRID[2]  # 4096
RADIUS = 0.25
R2 = RADIUS * RADIUS
K = 10
KPAD = 16
N = 32768
NCORES = 8
NQ = N // NCORES          # 4096 queries per core
NTILES = NQ // 128        # 32 tiles of 128 queries
NSEG = 8
SEG = G // NSEG           # 512

F32 = mybir.dt.float32
F32R = mybir.dt.float32r
BF16 = mybir.dt.bfloat16

USE_F32R_MATMUL = False
USE_GPSIMD_SPLIT = True
# Compute v = d2 - thr via a 38-row bf16-split matmul (sub-ulp of fp32)
# instead of the 4x-slower fp32 PE mode.
USE_SPLIT_MM = True
KROWS = 38
# Ball-widening margin: must exceed any |device d2 - reference fp32 d2|.
DELTA = 1e-5
# Columns of half 0 whose score pass stays on the DVE (rest: ACT+GPSIMD).
# 2048 = all of half 0; GPSIMD shares the DVE SBUF port, so pushing more
# work to it does not pay.
DVE_COLS = 2048

_cached = {}

# test.py hooks: set TRACE["on"]=True to profile; last results land in
# TRACE["last"].
TRACE = {"on": False, "last": None, "tmpdir": None}


def _build_kernel():
    nc = bacc.Bacc(None)

    # xr: [xT | rhs_db] stacked rows; tb: [thr | scorebase] on 128.
    if USE_SPLIT_MM:
        xr = nc.declare_dram_parameter("xr", [KROWS, NQ + G], BF16, isOutput=False)
    else:
        xr = nc.declare_dram_parameter("xr", [4, NQ + G], F32, isOutput=False)
    tb = nc.declare_dram_parameter("tb", [128, NTILES + G], F32, isOutput=False)
    out_vals = nc.declare_dram_parameter(
        "vals", [NTILES, 128, KPAD + 2], F32, isOutput=True
    )

    HALF = G // 2  # 2048: PSUM tile of 4 banks
    mmdt = F32R if USE_F32R_MATMUL else F32

    with tile.TileContext(nc) as tc:
        with (
            tc.tile_pool(name="consts", bufs=1) as consts,
            tc.tile_pool(name="psum", bufs=2, space="PSUM") as psum,
            tc.tile_pool(name="score", bufs=2) as scorep,
            tc.tile_pool(name="cand", bufs=2) as candp,
            tc.tile_pool(name="t16", bufs=4) as t16p,
            tc.tile_pool(name="d2sb", bufs=2) as d2sbp,
        ):
            if USE_SPLIT_MM:
                sb_xr = consts.tile([KROWS, NQ + G], BF16)
            else:
                sb_xr = consts.tile([4, NQ + G], F32)
            sb_tb = consts.tile([128, NTILES + G], F32)
            nc.sync.dma_start(sb_xr[:], xr[:])
            nc.sync.dma_start(sb_tb[:], tb[:])
            sb_eps = consts.tile([128, 1], F32)
            nc.vector.memset(sb_eps[:], DELTA)

            for t in range(NTILES):
                score = scorep.tile([128, G], F32)
                for h in range(2):
                    ps = psum.tile([128, HALF], F32)
                    for j in range(HALF // 512):
                        g0 = NQ + h * HALF + j * 512
                        if USE_SPLIT_MM:
                            lhsT = sb_xr[:, t * 128 : (t + 1) * 128]
                            rhs = sb_xr[:, g0 : g0 + 512]
                        else:
                            lhsT = sb_xr[:, t * 128 : (t + 1) * 128].bitcast(mmdt)
                            rhs = sb_xr[:, g0 : g0 + 512].bitcast(mmdt)
                        nc.tensor.matmul(
                            ps[:, j * 512 : (j + 1) * 512],
                            lhsT=lhsT,
                            rhs=rhs,
                            start=True,
                            stop=True,
                        )
                    # score = (v <= cut) * scorebase   (permuted G order)
                    # split-MM already folded thr in.  cut = +DELTA widens
                    # the ball so the device set is a superset of the
                    # reference's fp32 in-radius set regardless of rounding
                    # differences; the host filters candidates with
                    # reference-exact arithmetic afterwards.
                    cut = DELTA if USE_SPLIT_MM else sb_tb[:, t : t + 1]
                    b0 = NTILES + h * HALF
                    # DVE takes the first DVE_COLS columns of half 0; the
                    # rest go ScalarE sign + GPSIMD multiply.  s in
                    # {-1,0,+1}; negative scores lose to every positive
                    # score, so the selection is unchanged.  eps makes
                    # v == 0 inclusive (d2 == R^2 belongs in the ball).
                    dcols = DVE_COLS if (USE_GPSIMD_SPLIT and h == 0) else (
                        HALF if not USE_GPSIMD_SPLIT else 0
                    )
                    if dcols:
                        nc.vector.scalar_tensor_tensor(
                            out=score[:, h * HALF : h * HALF + dcols],
                            in0=ps[:, 0:dcols],
                            scalar=cut,
                            in1=sb_tb[:, b0 : b0 + dcols],
                            op0=mybir.AluOpType.is_le,
                            op1=mybir.AluOpType.mult,
                        )
                    if dcols < HALF:
                        w = HALF - dcols
                        d2sb = d2sbp.tile([128, w], BF16, tag="d2sb")
                        nc.scalar.activation(
                            d2sb[:],
                            ps[:, dcols:HALF],
                            mybir.ActivationFunctionType.Sign,
                            bias=sb_eps[:],
                            scale=-1.0,
                        )
                        nc.gpsimd.tensor_tensor(
                            out=score[:, h * HALF + dcols : (h + 1) * HALF],
                            in0=d2sb[:],
                            in1=sb_tb[:, b0 + dcols : b0 + HALF],
                            op=mybir.AluOpType.mult,
                        )
                # per-segment top-8 (segments are contiguous 512 slices in
                # the permuted order)
                cand = candp.tile([128, 64], F32)
                for s in range(NSEG):
                    nc.vector.max(
                        cand[:, 8 * s : 8 * s + 8],
                        score[:, SEG * s : SEG * (s + 1)],
                    )
                t16 = t16p.tile([128, KPAD + 2], F32)
                # global top-8 of the 64 candidates
                nc.vector.max(t16[:, 0:8], cand[:])
                # keep candidates strictly below the 8th value, then top-8
                # again -> ranks 9..16
                cand2 = candp.tile([128, 64], F32, tag="cand2")
                nc.vector.scalar_tensor_tensor(
                    out=cand2[:],
                    in0=cand[:],
                    scalar=t16[:, 7:8],
                    in1=cand[:],
                    op0=mybir.AluOpType.is_lt,
                    op1=mybir.AluOpType.mult,
                )
                nc.vector.max(t16[:, 8:16], cand2[:])
                # validity: segment's 8th value must not beat merged 16th
                segv8 = t16p.tile([128, 8], F32, tag="segv8")
                nc.vector.tensor_scalar(
                    out=segv8[:],
                    in0=cand[:, 7:64:8],
                    scalar1=t16[:, 15:16],
                    scalar2=0.0,
                    op0=mybir.AluOpType.is_gt,
                    op1=mybir.AluOpType.add,
                    accum_out=t16[:, 16:17],
                )
                nc.sync.dma_start(out_vals[t], t16[:])

    nc.finalize()
    return nc


def _split4(v):
    """4-way bf16 split: v ~= s0 + s1 + s2 + s3 to ~2^-33 relative."""
    import ml_dtypes

    bf = ml_dtypes.bfloat16
    out = []
    r = v.astype(np.float32)
    for _ in range(4):
        s = r.astype(bf).astype(np.float32)
        out.append(s)
        r = r - s
    return out


def _build_split_rows(xc, gperm, thr):
    """lhsT rows (KROWS, NQ) and rhs rows (KROWS, G) such that
    sum_k lhsT[k, q] * rhs[k, g] ~= d2(q, g) - thr(q)  (sub-ulp of fp32)."""
    NQl = xc.shape[0]
    lhs = np.zeros((KROWS, NQl), np.float32)
    rhs = np.zeros((KROWS, G), np.float32)
    # product pairs (i, j) with i+j <= 3 cover terms down to ~2^-32
    pairs = [(i, j) for i in range(4) for j in range(4) if i + j <= 3]
    r = 0
    for c in range(3):
        qs = _split4(xc[:, c])
        us = _split4(-2.0 * gperm[:, c])
        for i, j in pairs:
            lhs[r] = qs[i]
            rhs[r] = us[j]
            r += 1
    for sv in _split4((gperm * gperm).sum(axis=1).astype(np.float32)):
        lhs[r] = 1.0
        rhs[r] = sv
        r += 1
    for tv in _split4(-thr):
        lhs[r] = tv
        rhs[r] = 1.0
        r += 1
    assert r == KROWS, r
    return lhs, rhs


def _host_fallback(xq, grid_orig, q_idx):
    """Reference-exact first-K ball query for the flagged queries."""
    out_map = np.zeros((len(q_idx), K), np.int32)
    out_valid = np.zeros((len(q_idx), K), bool)
    for i, qi in enumerate(q_idx):
        d2 = ((xq[qi][None, :] - grid_orig) ** 2).sum(axis=1)
        hits = np.nonzero(d2 <= np.float32(R2))[0][:K]
        out_map[i, : len(hits)] = hits
        out_valid[i, : len(hits)] = True
    return out_map, out_valid


def kernel(x: np.ndarray, p_grid: np.ndarray):
    assert x.shape == (1, N, 3) and p_grid.shape == (1,) + GRID + (3,)
    grid = np.ascontiguousarray(p_grid.reshape(G, 3).astype(np.float32))
    xq = np.ascontiguousarray(x.reshape(N, 3).astype(np.float32))

    # Permute db points into 8 interleaved segments: column j holds original
    # index perm[j]; segment s = columns [512*s, 512*(s+1)) = indices
    # congruent to s (mod 8).
    perm = np.arange(G).reshape(SEG, NSEG).T.reshape(-1)  # perm[j] = orig idx
    gperm = grid[perm]

    # rhs rows: [-2*gx, -2*gy, -2*gz, |g|^2]  (permuted order)
    rhs_db = np.empty((4, G), np.float32)
    rhs_db[0:3] = -2.0 * gperm.T
    rhs_db[3] = (gperm * gperm).sum(axis=1)

    # scorebase row: G - orig_idx, in permuted column order
    scorebase = np.broadcast_to(
        (G - perm).astype(np.float32)[None, :], (128, G)
    ).copy()

    import ml_dtypes

    in_maps = []
    for c in range(NCORES):
        xc = xq[c * NQ : (c + 1) * NQ]  # (NQ, 3)
        thr = (R2 - (xc * xc).sum(axis=1)).astype(np.float32)
        if USE_SPLIT_MM:
            lhs, rhs = _build_split_rows(xc, gperm, thr)
            xrh = np.empty((KROWS, NQ + G), ml_dtypes.bfloat16)
            xrh[:, :NQ] = lhs.astype(ml_dtypes.bfloat16)
            xrh[:, NQ:] = rhs.astype(ml_dtypes.bfloat16)
        else:
            xrh = np.empty((4, NQ + G), np.float32)
            xrh[0:3, :NQ] = xc.T
            xrh[3, :NQ] = 1.0
            xrh[:, NQ:] = rhs_db
        thrt = thr.reshape(NTILES, 128).T  # (128, NTILES)
        tbh = np.empty((128, NTILES + G), np.float32)
        tbh[:, :NTILES] = thrt
        tbh[:, NTILES:] = scorebase
        in_maps.append({"xr": xrh, "tb": tbh})

    if "nc" not in _cached:
        _cached["nc"] = _build_kernel()
    kwargs = {}
    if TRACE["on"]:
        kwargs = dict(trace=True, tmpdir=TRACE["tmpdir"])
    res = None
    for attempt in range(3):
        try:
            res = run_bass_kernel_spmd(
                _cached["nc"], in_maps, list(range(NCORES)), **kwargs
            )
            break
        except Exception:
            if attempt == 2:
                raise
    TRACE["last"] = res

    vals = np.concatenate(
        [res.results[c]["vals"].reshape(NQ, KPAD + 2) for c in range(NCORES)],
        axis=0,
    )  # (N, 18)
    v16 = vals[:, :KPAD]
    valid16 = v16 > 0
    idx16 = np.where(valid16, (G - v16), 0.0).astype(np.int32)
    # Filter the (widened-ball) candidates with reference-exact fp32 math.
    gg = grid[idx16]  # (N, 16, 3)
    dx = xq[:, None, 0] - gg[:, :, 0]
    dy = xq[:, None, 1] - gg[:, :, 1]
    dz = xq[:, None, 2] - gg[:, :, 2]
    d2r = (dx * dx + dy * dy) + dz * dz  # fp32, reference add order
    keep = valid16 & (d2r <= np.float32(R2))
    order = np.argsort(~keep, axis=1, kind="stable")
    idx_s = np.take_along_axis(idx16, order, axis=1)[:, :K]
    keep_s = np.take_along_axis(keep, order, axis=1)[:, :K]
    mapping = np.where(keep_s, idx_s, 0).astype(np.int32)
    valid = keep_s
    # fallback: segment overflow, or widened-top-16 exhausted before K kept
    nkeep = keep.sum(axis=1)
    flagged = np.nonzero((vals[:, 16] > 0) | ((nkeep < K) & valid16[:, 15]))[0]
    if len(flagged):
        fmap, fvalid = _host_fallback(xq, grid, flagged)
        mapping[flagged] = fmap
        valid[flagged] = fvalid
    outputs = grid[mapping] * valid[..., None].astype(np.float32)
    return mapping[None], outputs[None]
